# revision 1
# baseline (speedup 1.0000x reference)
"""Trainium2 8-core kernel for causal GQA attention (nn_Attention_90967407329949).

Distribution: tensor-parallel over query heads (2 q-heads + their shared
kv-head per core). Each core computes its heads' QKV projections from the full
input, RoPE, causal attention; the cores AllGather the per-head attention
outputs (one collective per batch, overlapped with compute) and each core
computes a 256-column slice of the output projection. The host concatenates
the 8 column slices.

All matmuls run in bf16 (fp32 PSUM accumulation). head_scale is folded into Wo
rows on the host. Softmax skips the running-max (scores are O(1) for this
problem: |s|max ~ 7, exp never overflows fp32); the denominators come from a
ones-vector matmul accumulated alongside the attention*V matmuls.

The attention inner loop is ScalarE(exp)-throughput-bound, which would leave
the TensorE idle-cooling (HAM re-throttle) between attention matmuls. To keep
TensorE dense, the emission interleaves each batch's attention with the next
batch's QKV projections (and the last batch's attention with the first output
projection) at a few-matmuls granularity via generators.

Layouts (T suffix = transposed, feature dim on SBUF partitions):
  xt   [2048, 4096]   x^T (model dim, b*1024+n tokens), bf16
  wq   [128, 16, 256] Wq k-tiles: wq[p,t,m] = Wq[t*128+p, c*256+m], bf16
  wk/wv[128, 16, 128] same for this core's kv head, bf16
  wo   [128, 16, 256] (head_scale-folded) Wo k-tiles for this core's col slice
  cost/sint [128, 1024] rotary tables transposed; sint sign-folded
  mask [128, 2048]    4 causal masks for the 4 diagonal offsets
  out  [256, 4096]    (out @ Wo)^T column slice, bf16 (host upcasts)
"""

import numpy as np
import ml_dtypes

import concourse.bacc as bacc
import concourse.mybir as mybir
import concourse.tile as tile
from concourse.bass_utils import run_bass_kernel_spmd

BF16 = mybir.dt.bfloat16
F32 = mybir.dt.float32

N_CORES = 8
B = 4
N = 1024           # sequence length per batch
NT = B * N         # 4096 tokens
D = 2048           # model dim
DH = 128           # head dim
KT = D // 128      # 16 contraction k-tiles
SCALE = 1.0 / np.sqrt(DH)

_NC_CACHE = {}


def build_nc():
    if "nc" in _NC_CACHE:
        return _NC_CACHE["nc"]
    nc = bacc.Bacc("TRN2", target_bir_lowering=False, debug=False, num_devices=N_CORES)

    xt = nc.dram_tensor("xt", [D, NT], BF16, kind="ExternalInput")
    wq = nc.dram_tensor("wq", [128, KT, 256], BF16, kind="ExternalInput")
    wk = nc.dram_tensor("wk", [128, KT, 128], BF16, kind="ExternalInput")
    wv = nc.dram_tensor("wv", [128, KT, 128], BF16, kind="ExternalInput")
    wo = nc.dram_tensor("wo", [128, KT, 256], BF16, kind="ExternalInput")
    cost = nc.dram_tensor("cost", [128, N], BF16, kind="ExternalInput")
    sint = nc.dram_tensor("sint", [128, N], BF16, kind="ExternalInput")
    mask = nc.dram_tensor("mask", [128, 2048], BF16, kind="ExternalInput")
    out = nc.dram_tensor("out", [256, NT], BF16, kind="ExternalOutput")

    # AllGather buffers, one pair of SEPARATE tensors per batch: Tile's DRAM
    # dependency tracking is tensor-granular, so a shared tensor would make
    # batch b+1's attention writes falsely wait on batch b's collective read
    # (blocking the whole SP ring behind them). Per-rank input rows are this
    # core's two heads; rank-major concat yields global head order directly.
    ag_in = [nc.dram_tensor(f"ag_in{p}", [256, 2 * N], BF16) for p in range(2)]
    ag_out = [nc.dram_tensor(f"ag_out{p}", [D, 2 * N], BF16, addr_space="Shared")
              for p in range(2)]
    ag_in_s = {b: nc.dram_tensor(f"ag_ins{b}", [256, N], BF16) for b in (2, 3)}
    ag_out_s = {b: nc.dram_tensor(f"ag_outs{b}", [D, N], BF16, addr_space="Shared")
                for b in (2, 3)}

    with tile.TileContext(nc) as tc:
        with (
            tc.tile_pool(name="const", bufs=1) as constp,
            tc.tile_pool(name="persist", bufs=1) as persist,
            tc.tile_pool(name="xtp", bufs=3) as xtp,
            tc.tile_pool(name="qkraw", bufs=2) as qkrawp,
            tc.tile_pool(name="rope", bufs=2) as ropep,
            tc.tile_pool(name="ep", bufs=4) as ep,
            tc.tile_pool(name="etmpp", bufs=2) as etmpp,
            tc.tile_pool(name="attp", bufs=3) as attp,
            tc.tile_pool(name="recipp", bufs=2) as recipp,
            tc.tile_pool(name="rbcp", bufs=2) as rbcp,
            tc.tile_pool(name="gp", bufs=3) as gp,
            tc.tile_pool(name="oobp", bufs=2) as oobp,
            tc.tile_pool(name="psacc", bufs=3, space="PSUM") as psacc,
            tc.tile_pool(name="pss", bufs=2, space="PSUM") as pss,
            tc.tile_pool(name="psu", bufs=2, space="PSUM") as psu,
            tc.tile_pool(name="pssum", bufs=1, space="PSUM") as pssum,
        ):
            # ---- constants ----
            wq_sb = constp.tile([128, KT, 256], BF16)
            wk_sb = constp.tile([128, KT, 128], BF16)
            wv_sb = constp.tile([128, KT, 128], BF16)
            wo_sb = constp.tile([128, KT, 256], BF16)
            cos_sb = constp.tile([128, N], BF16)
            sin_sb = constp.tile([128, N], BF16)
            mask_sb = constp.tile([128, 2048], BF16)
            ones_sb = constp.tile([128, 1], BF16)
            for c in range(4):  # chunked so the first matmuls start early
                nc.scalar.dma_start(wq_sb[:, c * 4:(c + 1) * 4, :],
                                    wq[:, c * 4:(c + 1) * 4, :])
            nc.scalar.dma_start(wk_sb[:], wk[:])
            nc.scalar.dma_start(wv_sb[:], wv[:])
            nc.vector.memset(ones_sb[:], 1.0)

            def late_consts():
                nc.scalar.dma_start(wo_sb[:], wo[:])
                nc.scalar.dma_start(cos_sb[:], cost[:])
                nc.scalar.dma_start(sin_sb[:], sint[:])
                nc.scalar.dma_start(mask_sb[:], mask[:])

            # ---- persistent per-core QKV (RoPE'd, transposed layouts) ----
            q_sb = [persist.tile([128, NT], BF16, name=f"q{h}_sb") for h in range(2)]
            k_sb = persist.tile([128, NT], BF16)
            v_sb = persist.tile([128, NT], BF16)  # 32 [tok,128]x[d,128] tiles

            xt_r = xt.rearrange("(t p) n -> p t n", p=128)

            def xblk_load(nb):
                col0 = nb * 512
                xblk = xtp.tile([128, KT, 512], BF16, tag="xblk", name=f"xblk_{nb}")
                ring = nc.sync if nb % 2 == 0 else nc.scalar
                if nb == 0:
                    # finer granularity so the first matmuls start early
                    for kt in range(KT):
                        ring.dma_start(xblk[:, kt, :],
                                       xt_r[:, kt, col0:col0 + 512])
                else:
                    ring.dma_start(xblk[:], xt_r[:, :, col0:col0 + 512])
                return xblk

            def rope_chunk(raw, dst, c0, col0):
                """RoPE 512 positions (table cols c0..c0+512) into dst at col0."""
                rot = ropep.tile([128, 512], BF16, tag="rot")
                nc.sync.dma_start(rot[0:64, :], raw[64:128, c0:c0 + 512])
                nc.sync.dma_start(rot[64:128, :], raw[0:64, c0:c0 + 512])
                t1 = ropep.tile([128, 512], BF16, tag="t1")
                nc.vector.tensor_mul(t1[:], raw[:, c0:c0 + 512],
                                     cos_sb[:, c0:c0 + 512])
                t2 = ropep.tile([128, 512], BF16, tag="t2")
                nc.vector.tensor_mul(t2[:], rot[:], sin_sb[:, c0:c0 + 512])
                nc.vector.tensor_add(dst[:, col0:col0 + 512], t1[:], t2[:])

            def qkv_gen(b, xblks=None):
                """Projections+RoPE for batch b, yielding between matmul chunks."""
                raw = [
                    qkrawp.tile([128, N], BF16, tag="qraw0", name=f"qraw0_{b}"),
                    qkrawp.tile([128, N], BF16, tag="qraw1", name=f"qraw1_{b}"),
                    qkrawp.tile([128, N], BF16, tag="kraw", name=f"kraw_{b}"),
                ]
                if xblks is None:
                    xblks = [xblk_load(2 * b), xblk_load(2 * b + 1)]
                if b == 0:
                    late_consts()
                for half, xblk in enumerate(xblks):
                    nb = 2 * b + half
                    col0 = nb * 512
                    c0 = half * 512
                    def accum(dst_ps, w_sb, msl):
                        for k0 in range(0, KT, 4):
                            for kt in range(k0, k0 + 4):
                                nc.tensor.matmul(
                                    dst_ps, w_sb[:, kt, msl], xblk[:, kt, :],
                                    start=(kt == 0), stop=(kt == KT - 1))
                            yield

                    # Q (2 head-tiles)
                    for m in range(2):
                        q_ps = psacc.tile([128, 512], F32, tag="psacc",
                                          name=f"q_ps_{nb}_{m}")
                        yield from accum(q_ps[:], wq_sb,
                                         slice(m * 128, (m + 1) * 128))
                        nc.scalar.activation(raw[m][:, c0:c0 + 512], q_ps[:],
                                             mybir.ActivationFunctionType.Copy)
                        yield
                    k_ps = psacc.tile([128, 512], F32, tag="psacc",
                                      name=f"k_ps_{nb}")
                    yield from accum(k_ps[:], wk_sb, slice(0, 128))
                    nc.scalar.activation(raw[2][:, c0:c0 + 512], k_ps[:],
                                         mybir.ActivationFunctionType.Copy)
                    yield
                    v_ps = psacc.tile([128, 512], F32, tag="psacc",
                                      name=f"v_ps_{nb}")
                    yield from accum(v_ps[:], wv_sb, slice(0, 128))
                    vraw = ropep.tile([128, 512], BF16, tag="vraw")
                    nc.scalar.activation(vraw[:], v_ps[:],
                                         mybir.ActivationFunctionType.Copy)
                    yield
                    # RoPE this block's positions; v transposes to [tok, d]
                    rope_chunk(raw[0], q_sb[0], c0, col0)
                    rope_chunk(raw[1], q_sb[1], c0, col0)
                    rope_chunk(raw[2], k_sb, c0, col0)
                    for i in range(4):
                        tt = nb * 4 + i
                        nc.sync.dma_start_transpose(
                            v_sb[:, tt * 128:(tt + 1) * 128],
                            vraw[:, i * 128:(i + 1) * 128])
                    yield

            def att_gen(b):
                """Attention for batch b, yielding between j-tile units."""
                for h in range(2):
                    qh = q_sb[h]
                    att = attp.tile([128, 1024], BF16, tag="att",
                                    name=f"att_{b}_{h}")
                    for ib in range(2):
                        icol = b * N + ib * 512
                        cnt = 4 * ib + 4
                        u_ps = psu.tile([128, 512], F32, tag="psu",
                                        name=f"u_ps_{b}_{h}_{ib}")
                        sum_ps = pssum.tile([1, 512], F32, tag="pssum",
                                            name=f"sum_ps_{b}_{h}_{ib}")

                        def c_lo(jt):
                            # diagonal tile at offset r: columns < 128*r are
                            # causally invalid for every row -- skip them in
                            # every consumer (exact: those (j,i) pairs are
                            # fully masked, and sum/u accumulation over the
                            # remaining tiles covers the kept columns).
                            r = jt - 4 * ib
                            return 128 * r if r > 0 else 0

                        def s_mm(jt):
                            s_ps = pss.tile([128, 512], F32, tag="pss",
                                            name=f"s_ps_{b}_{h}_{ib}_{jt}")
                            jcol = b * N + jt * 128
                            c0 = c_lo(jt)
                            nc.tensor.matmul(
                                s_ps[:, c0:512], k_sb[:, jcol:jcol + 128],
                                qh[:, icol + c0:icol + 512],
                                start=True, stop=True)
                            return s_ps

                        def e_of(jt, s_ps):
                            r = jt - 4 * ib
                            c0 = c_lo(jt)
                            e = ep.tile([128, 512], BF16, tag="e",
                                        name=f"e_{b}_{h}_{ib}_{jt}")
                            if r >= 0:  # diagonal tile: mask after exp
                                etmp = etmpp.tile([128, 512], BF16, tag="etmp")
                                nc.scalar.activation(
                                    etmp[:, c0:512], s_ps[:, c0:512],
                                    mybir.ActivationFunctionType.Exp, scale=SCALE)
                                nc.vector.tensor_mul(
                                    e[:, c0:512], etmp[:, c0:512],
                                    mask_sb[:, r * 512 + c0:(r + 1) * 512])
                            else:
                                nc.scalar.activation(
                                    e[:], s_ps[:],
                                    mybir.ActivationFunctionType.Exp, scale=SCALE)
                            return e

                        s_tiles = {0: s_mm(0), 1: s_mm(1)}
                        for jt in range(cnt):
                            e = e_of(jt, s_tiles.pop(jt))
                            if jt + 2 < cnt:
                                s_tiles[jt + 2] = s_mm(jt + 2)
                            tt = b * 8 + jt
                            c0 = c_lo(jt)
                            nc.tensor.matmul(
                                u_ps[:, c0:512],
                                v_sb[:, tt * 128:(tt + 1) * 128], e[:, c0:512],
                                start=(jt == 0), stop=(jt == cnt - 1),
                                skip_group_check=True)
                            nc.tensor.matmul(
                                sum_ps[:, c0:512], ones_sb[:], e[:, c0:512],
                                start=(jt == 0), stop=(jt == cnt - 1),
                                skip_group_check=True)
                            yield
                        recip = recipp.tile([1, 512], F32, tag="recip")
                        nc.vector.reciprocal_approx_fast(out=recip[:], in_=sum_ps[:])
                        rbc = rbcp.tile([128, 512], F32, tag="rbc")
                        nc.gpsimd.partition_broadcast(rbc[:], recip[:])
                        nc.vector.tensor_mul(
                            att[:, ib * 512:(ib + 1) * 512], u_ps[:], rbc[:])
                        yield
                    if b < 2:
                        nc.sync.dma_start(
                            ag_in[0][h * 128:(h + 1) * 128,
                                     b * N:(b + 1) * N], att[:])
                    else:
                        nc.sync.dma_start(
                            ag_in_s[b][h * 128:(h + 1) * 128], att[:])

            def allgather_pair(p):
                nc.gpsimd.collective_compute(
                    "AllGather",
                    mybir.AluOpType.bypass,
                    replica_groups=[list(range(N_CORES))],
                    ins=[ag_in[p][:].opt()],
                    outs=[ag_out[p][:].opt()],
                )

            def allgather_single(b):
                nc.gpsimd.collective_compute(
                    "AllGather",
                    mybir.AluOpType.bypass,
                    replica_groups=[list(range(N_CORES))],
                    ins=[ag_in_s[b][:].opt()],
                    outs=[ag_out_s[b][:].opt()],
                )

            ag_out_r = [t.rearrange("(t p) n -> p t n", p=128) for t in ag_out]
            ag_out_sr = {b: t.rearrange("(t p) n -> p t n", p=128)
                         for b, t in ag_out_s.items()}

            g_tiles = {}

            def g_prefetch(b, ib, ring):
                """Load one gathered [2048, 512] slab of batch b for oproj."""
                g_tiles[(b, ib)] = gp.tile([128, KT, 512], BF16, tag="g",
                                           name=f"g_{b}_{ib}")
                if b < 2:
                    col = b * N + ib * 512
                    src = ag_out_r[0][:, :, col:col + 512]
                else:
                    src = ag_out_sr[b][:, :, ib * 512:(ib + 1) * 512]
                ring.dma_start(g_tiles[(b, ib)][:], src)

            def oproj_gen(b):
                osb = [oobp.tile([128, 1024], BF16, tag="osb", name=f"osb_{b}_{m}")
                       for m in range(2)]
                for ib in range(2):
                    g = g_tiles.pop((b, ib))
                    for m in range(2):
                        o_ps = psacc.tile([128, 512], F32, tag="psacc",
                                          name=f"o_ps_{b}_{ib}_{m}")
                        for k0 in range(0, KT, 4):
                            for kt in range(k0, k0 + 4):
                                nc.tensor.matmul(
                                    o_ps[:], wo_sb[:, kt, m * 128:(m + 1) * 128],
                                    g[:, kt, :], start=(kt == 0),
                                    stop=(kt == KT - 1))
                            yield
                        nc.vector.tensor_copy(
                            osb[m][:, ib * 512:(ib + 1) * 512], o_ps[:])
                        yield
                for m in range(2):
                    nc.sync.dma_start(
                        out[m * 128:(m + 1) * 128, b * N:(b + 1) * N], osb[m][:])

            def drain(gen):
                for _ in gen:
                    pass

            def interleave(gen_a, gen_b, ratio_a=2):
                """Alternate generators, taking ratio_a steps of gen_a per
                step of gen_b: attention (gen_a) finishes early in the
                segment, so its AllGather triggers sooner."""
                alive = [gen_a, gen_b]
                while alive:
                    for g in list(alive):
                        steps = ratio_a if g is gen_a else 1
                        for _ in range(steps):
                            try:
                                next(g)
                            except StopIteration:
                                if g in alive:
                                    alive.remove(g)
                                break

            # Pipeline: attention(b) (ScalarE-bound) interleaved with the next
            # batch's projections (TensorE-bound) so TensorE stays dense and
            # HAM-warm; per-batch AllGathers spread across the run; trailing
            # output projections covered by completed AllGathers.
            drain(qkv_gen(0))
            interleave(att_gen(0), qkv_gen(1))
            interleave(att_gen(1), qkv_gen(2))
            # input loads for the post-collective interleave are emitted
            # BEFORE the collective: Tile's cumulative DMA-lane semaphores
            # make any DMA consumer emitted after a collective transitively
            # wait for it.
            xblk67 = [xblk_load(6), xblk_load(7)]
            allgather_pair(0)
            interleave(att_gen(2), qkv_gen(3, xblk67))
            allgather_single(2)
            g_prefetch(0, 0, nc.sync)
            g_prefetch(0, 1, nc.scalar)
            interleave(att_gen(3), oproj_gen(0))
            allgather_single(3)
            g_prefetch(1, 0, nc.sync)
            g_prefetch(1, 1, nc.scalar)
            drain(oproj_gen(1))
            g_prefetch(2, 0, nc.sync)
            g_prefetch(2, 1, nc.scalar)
            drain(oproj_gen(2))
            g_prefetch(3, 0, nc.sync)
            g_prefetch(3, 1, nc.scalar)
            drain(oproj_gen(3))

    nc.compile()
    _NC_CACHE["nc"] = nc
    return nc


def _host_prep(x, Wq, Wk, Wv, Wo, head_scale):
    bf = ml_dtypes.bfloat16
    xt = np.ascontiguousarray(x.reshape(NT, D).T).astype(bf)

    hs = np.asarray(head_scale).reshape(16)
    wo_s = (np.asarray(Wo) * np.repeat(hs, DH)[:, None]).astype(np.float32)

    def ktile(w):  # [2048, M] -> [128, 16, M]
        m = w.shape[1]
        return np.ascontiguousarray(
            w.reshape(KT, 128, m).transpose(1, 0, 2)).astype(bf)

    inv_freq = (1.0 / (10000.0 ** (np.arange(0, DH, 2, dtype=np.float64) / DH)))
    freqs = np.arange(N, dtype=np.float64)[:, None] * inv_freq[None, :]  # [N, 64]
    emb = np.concatenate([freqs, freqs], axis=-1)  # [N, 128]
    cosT = np.ascontiguousarray(np.cos(emb).T).astype(bf)  # [128, N]
    sinT = np.sin(emb).T  # [128, N]
    sign = np.where(np.arange(DH) < 64, -1.0, 1.0)[:, None]
    sinT = np.ascontiguousarray(sinT * sign).astype(bf)

    # 4 diagonal masks r=0..3: valid (c >= p + 128*r)
    p = np.arange(128)[:, None]
    c = np.arange(512)[None, :]
    masks = [(c >= p + 128 * r).astype(np.float32) for r in range(4)]
    mask = np.concatenate(masks, axis=1).astype(bf)  # [128, 2048]

    in_maps = []
    for core in range(N_CORES):
        kv = core // 2
        in_maps.append({
            "xt": xt,
            "wq": ktile(np.asarray(Wq)[:, core * 256:(core + 1) * 256]),
            "wk": ktile(np.asarray(Wk)[:, kv * 128:(kv + 1) * 128]),
            "wv": ktile(np.asarray(Wv)[:, kv * 128:(kv + 1) * 128]),
            "wo": ktile(wo_s[:, core * 256:(core + 1) * 256]),
            "cost": cosT,
            "sint": sinT,
            "mask": mask,
        })
    return in_maps


def kernel(x, Wq, Wk, Wv, Wo, head_scale, _run_kwargs=None):
    nc = build_nc()
    in_maps = _host_prep(x, Wq, Wk, Wv, Wo, head_scale)
    res = run_bass_kernel_spmd(
        nc, in_maps, core_ids=list(range(N_CORES)), **(_run_kwargs or {})
    )
    outT = np.concatenate(
        [res.results[c]["out"].astype(np.float32) for c in range(N_CORES)], axis=0)
    full = np.ascontiguousarray(outT.T).reshape(B, N, D)
    if _run_kwargs:
        kernel.last_results = res
    return full



# revision 2
# speedup vs baseline: 1.6741x; 1.6741x over previous
"""Trainium2 8-core kernel for causal GQA attention (nn_Attention_90967407329949).

Distribution: (batch x head-group) data parallel with host-side reduce — zero
device collectives. Core (b, g) handles batch b (1024 tokens) and q-heads
g*8..g*8+7 (plus their 2 kv heads): it computes Q/K/V projections for its
heads over the full batch, RoPE, full causal attention, and a PARTIAL output
projection (contraction over its 8 heads' dims of Wo). The host sums the two
partials per batch. Vs. tensor-parallel + AllGather this removes all
collectives (and their DMA-queue poisoning) and halves K/V projection work.

All matmuls run in bf16 (fp32 PSUM accumulation). head_scale is folded into Wo
rows on the host. Softmax skips the running-max (scores are O(1) here: |s|max
~ 4 after scaling, exp never overflows); denominators come from a ones-vector
matmul accumulated alongside the attention*V matmuls.

The attention inner loop is ScalarE(exp)-heavy, so attention for head h is
interleaved with the Q projection of head h+2 (TensorE-bound) at a
few-matmuls granularity via generators to keep TensorE dense.

Layouts (feature dim on SBUF partitions):
  xb   [128, 16, 1024]  x^T k-tiles for this batch, bf16
  wq   [128, 16, 1024]  Wq k-tiles, this core's 8 heads
  wk/wv[128, 16, 256]   this core's 2 kv heads
  wo   [128, 8, 2048]   (head_scale-folded) Wo row-tiles for its 8 heads
  cost/sint [128, 1024] rotary tables transposed; sint sign-folded
  mask [128, 2048]      4 causal masks for the 4 diagonal offsets
  out  [2048, 1024]     partial (out @ Wo)^T for this batch, bf16
"""

import numpy as np
import ml_dtypes

import concourse.bacc as bacc
import concourse.mybir as mybir
import concourse.tile as tile
from concourse.bass_utils import run_bass_kernel_spmd

BF16 = mybir.dt.bfloat16
F32 = mybir.dt.float32

N_CORES = 8
B = 4
N = 1024           # sequence length per batch
D = 2048           # model dim
DH = 128           # head dim
KT = D // 128      # 16 contraction k-tiles
NH = 8             # q heads per core
NKV = 2            # kv heads per core
SCALE = 1.0 / np.sqrt(DH)

_NC_CACHE = {}


def build_nc():
    if "nc" in _NC_CACHE:
        return _NC_CACHE["nc"]
    nc = bacc.Bacc("TRN2", target_bir_lowering=False, debug=False, num_devices=N_CORES)

    xb = nc.dram_tensor("xb", [128, KT, N], BF16, kind="ExternalInput")
    wq = nc.dram_tensor("wq", [128, KT, NH * 128], BF16, kind="ExternalInput")
    wk = nc.dram_tensor("wk", [128, KT, NKV * 128], BF16, kind="ExternalInput")
    wv = nc.dram_tensor("wv", [128, KT, NKV * 128], BF16, kind="ExternalInput")
    wo = nc.dram_tensor("wo", [128, NH, D], BF16, kind="ExternalInput")
    cost = nc.dram_tensor("cost", [128, N], BF16, kind="ExternalInput")
    sint = nc.dram_tensor("sint", [128, N], BF16, kind="ExternalInput")
    mask = nc.dram_tensor("mask", [128, 2048], BF16, kind="ExternalInput")
    out = nc.dram_tensor("out", [D, N], BF16, kind="ExternalOutput")

    with tile.TileContext(nc) as tc:
        with (
            tc.tile_pool(name="const", bufs=1) as constp,
            tc.tile_pool(name="persist", bufs=1) as persist,
            tc.tile_pool(name="qkraw", bufs=2) as qkrawp,
            tc.tile_pool(name="rope", bufs=2) as ropep,
            tc.tile_pool(name="ep", bufs=4) as ep,
            tc.tile_pool(name="etmpp", bufs=2) as etmpp,
            tc.tile_pool(name="recipp", bufs=2) as recipp,
            tc.tile_pool(name="rbcp", bufs=2) as rbcp,
            tc.tile_pool(name="oobp", bufs=3) as oobp,
            tc.tile_pool(name="psacc", bufs=3, space="PSUM") as psacc,
            tc.tile_pool(name="pss", bufs=2, space="PSUM") as pss,
            tc.tile_pool(name="psu", bufs=2, space="PSUM") as psu,
            tc.tile_pool(name="pssum", bufs=1, space="PSUM") as pssum,
        ):
            # ---- constants ----
            xb_sb = constp.tile([128, KT, N], BF16)
            wq_sb = constp.tile([128, KT, NH * 128], BF16)
            wk_sb = constp.tile([128, KT, NKV * 128], BF16)
            wv_sb = constp.tile([128, KT, NKV * 128], BF16)
            wo_sb = constp.tile([128, NH, D], BF16)
            cos_sb = constp.tile([128, N], BF16)
            sin_sb = constp.tile([128, N], BF16)
            mask_sb = constp.tile([128, 2048], BF16)
            ones_sb = constp.tile([128, 1], BF16)
            # x first, k-tile-granular so the first projections start early
            for kt in range(KT):
                nc.sync.dma_start(xb_sb[:, kt, :], xb[:, kt, :])
            nc.scalar.dma_start(wk_sb[:], wk[:])
            nc.scalar.dma_start(wv_sb[:], wv[:])
            nc.vector.memset(ones_sb[:], 1.0)
            for c in range(4):
                nc.scalar.dma_start(wq_sb[:, c * 4:(c + 1) * 4, :],
                                    wq[:, c * 4:(c + 1) * 4, :])

            def late_consts():
                nc.scalar.dma_start(cos_sb[:], cost[:])
                nc.scalar.dma_start(sin_sb[:], sint[:])
                nc.scalar.dma_start(mask_sb[:], mask[:])
                for c in range(4):
                    nc.scalar.dma_start(wo_sb[:, c * 2:(c + 1) * 2, :],
                                        wo[:, c * 2:(c + 1) * 2, :])

            # ---- persistent per-core tensors ----
            k_sb = persist.tile([128, NKV * N], BF16)   # RoPE'd K, [d, kv*1024]
            v_sb = persist.tile([128, NKV * N], BF16)   # 8 [tok,128] tiles per kv
            q_sb = [persist.tile([128, N], BF16, name=f"q{h}_sb") for h in range(NH)]
            att_sb = [persist.tile([128, N], BF16, name=f"att{h}_sb")
                      for h in range(NH)]

            def rope_chunk(raw, dst, c0):
                """RoPE 512 positions (cols c0..c0+512) of raw into dst."""
                rot = ropep.tile([128, 512], BF16, tag="rot")
                nc.sync.dma_start(rot[0:64, :], raw[64:128, c0:c0 + 512])
                nc.sync.dma_start(rot[64:128, :], raw[0:64, c0:c0 + 512])
                t1 = ropep.tile([128, 512], BF16, tag="t1")
                nc.vector.tensor_mul(t1[:], raw[:, c0:c0 + 512],
                                     cos_sb[:, c0:c0 + 512])
                t2 = ropep.tile([128, 512], BF16, tag="t2")
                nc.vector.tensor_mul(t2[:], rot[:], sin_sb[:, c0:c0 + 512])
                nc.vector.tensor_add(dst[:, c0:c0 + 512], t1[:], t2[:])

            def accum(dst_ps, w_sb, msl, csl):
                for k0 in range(0, KT, 4):
                    for kt in range(k0, k0 + 4):
                        nc.tensor.matmul(
                            dst_ps, w_sb[:, kt, msl], xb_sb[:, kt, csl],
                            start=(kt == 0), stop=(kt == KT - 1))
                    yield

            def kv_gen():
                """K and V projections + RoPE(K) + transpose(V), both kv heads."""
                late_consts()
                for kv in range(NKV):
                    kraw = qkrawp.tile([128, N], BF16, tag="kraw",
                                       name=f"kraw_{kv}")
                    for c in range(2):
                        k_ps = psacc.tile([128, 512], F32, tag="psacc",
                                          name=f"k_ps_{kv}_{c}")
                        yield from accum(k_ps[:], wk_sb,
                                         slice(kv * 128, (kv + 1) * 128),
                                         slice(c * 512, (c + 1) * 512))
                        nc.scalar.activation(kraw[:, c * 512:(c + 1) * 512],
                                             k_ps[:],
                                             mybir.ActivationFunctionType.Copy)
                        yield
                    rope_chunk(kraw, k_sb[:, kv * N:(kv + 1) * N], 0)
                    rope_chunk(kraw, k_sb[:, kv * N:(kv + 1) * N], 512)
                    yield
                for kv in range(NKV):
                    for c in range(2):
                        v_ps = psacc.tile([128, 512], F32, tag="psacc",
                                          name=f"v_ps_{kv}_{c}")
                        yield from accum(v_ps[:], wv_sb,
                                         slice(kv * 128, (kv + 1) * 128),
                                         slice(c * 512, (c + 1) * 512))
                        vraw = ropep.tile([128, 512], BF16, tag="vraw")
                        nc.scalar.activation(vraw[:], v_ps[:],
                                             mybir.ActivationFunctionType.Copy)
                        for i in range(4):
                            tt = kv * 8 + c * 4 + i
                            nc.sync.dma_start_transpose(
                                v_sb[:, tt * 128:(tt + 1) * 128],
                                vraw[:, i * 128:(i + 1) * 128])
                        yield

            def qproj_gen(h):
                """Q projection + RoPE for head h."""
                qraw = qkrawp.tile([128, N], BF16, tag="qraw", name=f"qraw_{h}")
                for c in range(2):
                    q_ps = psacc.tile([128, 512], F32, tag="psacc",
                                      name=f"q_ps_{h}_{c}")
                    yield from accum(q_ps[:], wq_sb,
                                     slice(h * 128, (h + 1) * 128),
                                     slice(c * 512, (c + 1) * 512))
                    nc.scalar.activation(qraw[:, c * 512:(c + 1) * 512], q_ps[:],
                                         mybir.ActivationFunctionType.Copy)
                    yield
                rope_chunk(qraw, q_sb[h], 0)
                rope_chunk(qraw, q_sb[h], 512)
                yield

            def att_gen(h):
                """Causal attention for head h, yielding between j-tile units."""
                kv = h // 4
                qh = q_sb[h]
                att = att_sb[h]
                for ib in range(2):
                    icol = ib * 512
                    cnt = 4 * ib + 4
                    u_ps = psu.tile([128, 512], F32, tag="psu",
                                    name=f"u_ps_{h}_{ib}")
                    sum_ps = pssum.tile([1, 512], F32, tag="pssum",
                                        name=f"sum_ps_{h}_{ib}")

                    def c_lo(jt):
                        # diagonal tile at offset r: columns < 128*r are
                        # causally invalid for every row -- skip them in
                        # every consumer (exact: those (j,i) pairs are
                        # fully masked).
                        r = jt - 4 * ib
                        return 128 * r if r > 0 else 0

                    def s_mm(jt):
                        s_ps = pss.tile([128, 512], F32, tag="pss",
                                        name=f"s_ps_{h}_{ib}_{jt}")
                        jcol = kv * N + jt * 128
                        c0 = c_lo(jt)
                        nc.tensor.matmul(
                            s_ps[:, c0:512], k_sb[:, jcol:jcol + 128],
                            qh[:, icol + c0:icol + 512],
                            start=True, stop=True)
                        return s_ps

                    def e_of(jt, s_ps):
                        r = jt - 4 * ib
                        c0 = c_lo(jt)
                        e = ep.tile([128, 512], BF16, tag="e",
                                    name=f"e_{h}_{ib}_{jt}")
                        if r >= 0:  # diagonal tile: mask after exp
                            etmp = etmpp.tile([128, 512], BF16, tag="etmp")
                            nc.scalar.activation(
                                etmp[:, c0:512], s_ps[:, c0:512],
                                mybir.ActivationFunctionType.Exp, scale=SCALE)
                            nc.vector.tensor_mul(
                                e[:, c0:512], etmp[:, c0:512],
                                mask_sb[:, r * 512 + c0:(r + 1) * 512])
                        else:
                            nc.scalar.activation(
                                e[:], s_ps[:],
                                mybir.ActivationFunctionType.Exp, scale=SCALE)
                        return e

                    s_tiles = {0: s_mm(0), 1: s_mm(1)}
                    for jt in range(cnt):
                        e = e_of(jt, s_tiles.pop(jt))
                        if jt + 2 < cnt:
                            s_tiles[jt + 2] = s_mm(jt + 2)
                        tt = kv * 8 + jt
                        c0 = c_lo(jt)
                        nc.tensor.matmul(
                            u_ps[:, c0:512],
                            v_sb[:, tt * 128:(tt + 1) * 128], e[:, c0:512],
                            start=(jt == 0), stop=(jt == cnt - 1),
                            skip_group_check=True)
                        nc.tensor.matmul(
                            sum_ps[:, c0:512], ones_sb[:], e[:, c0:512],
                            start=(jt == 0), stop=(jt == cnt - 1),
                            skip_group_check=True)
                        yield
                    recip = recipp.tile([1, 512], F32, tag="recip")
                    nc.vector.reciprocal_approx_fast(out=recip[:], in_=sum_ps[:])
                    rbc = rbcp.tile([128, 512], F32, tag="rbc")
                    nc.gpsimd.partition_broadcast(rbc[:], recip[:])
                    nc.vector.tensor_mul(
                        att[:, ib * 512:(ib + 1) * 512], u_ps[:], rbc[:])
                    yield

            def oproj_gen():
                """Partial output projection: contraction over this core's
                8 heads; one [128,1024] slab per output m-tile."""
                for m in range(16):
                    osb = oobp.tile([128, N], BF16, tag="osb", name=f"osb_{m}")
                    for c in range(2):
                        o_ps = psacc.tile([128, 512], F32, tag="psacc",
                                          name=f"o_ps_{m}_{c}")
                        for hh in range(NH):
                            nc.tensor.matmul(
                                o_ps[:], wo_sb[:, hh, m * 128:(m + 1) * 128],
                                att_sb[hh][:, c * 512:(c + 1) * 512],
                                start=(hh == 0), stop=(hh == NH - 1))
                            if hh % 4 == 3:
                                yield
                        nc.scalar.activation(osb[:, c * 512:(c + 1) * 512],
                                             o_ps[:],
                                             mybir.ActivationFunctionType.Copy)
                        yield
                    nc.sync.dma_start(out[m * 128:(m + 1) * 128, :], osb[:])

            def drain(gen):
                for _ in gen:
                    pass

            def interleave(gen_a, gen_b, ratio_a=1):
                alive = [gen_a, gen_b]
                while alive:
                    for g in list(alive):
                        steps = ratio_a if g is gen_a else 1
                        for _ in range(steps):
                            try:
                                next(g)
                            except StopIteration:
                                if g in alive:
                                    alive.remove(g)
                                break

            # Pipeline: K/V+Q(0,1) projections run dense; attention(h)
            # (ScalarE-heavy) interleaves with Q projection of head h+2
            # (TensorE-bound); trailing heads run alone; output projection
            # is one dense matmul run at the end.
            drain(kv_gen())
            drain(qproj_gen(0))
            drain(qproj_gen(1))
            for h in range(NH):
                if h + 2 < NH:
                    interleave(att_gen(h), qproj_gen(h + 2))
                else:
                    drain(att_gen(h))
            drain(oproj_gen())

    nc.compile()
    _NC_CACHE["nc"] = nc
    return nc


def _host_prep(x, Wq, Wk, Wv, Wo, head_scale):
    bf = ml_dtypes.bfloat16

    hs = np.asarray(head_scale).reshape(16)
    wo_s = (np.asarray(Wo) * np.repeat(hs, DH)[:, None]).astype(np.float32)

    def ktile(w):  # [2048, M] -> [128, 16, M]
        m = w.shape[1]
        return np.ascontiguousarray(
            w.reshape(KT, 128, m).transpose(1, 0, 2)).astype(bf)

    inv_freq = (1.0 / (10000.0 ** (np.arange(0, DH, 2, dtype=np.float64) / DH)))
    freqs = np.arange(N, dtype=np.float64)[:, None] * inv_freq[None, :]  # [N, 64]
    emb = np.concatenate([freqs, freqs], axis=-1)  # [N, 128]
    cosT = np.ascontiguousarray(np.cos(emb).T).astype(bf)  # [128, N]
    sinT = np.sin(emb).T  # [128, N]
    sign = np.where(np.arange(DH) < 64, -1.0, 1.0)[:, None]
    sinT = np.ascontiguousarray(sinT * sign).astype(bf)

    # 4 diagonal masks r=0..3: valid (c >= p + 128*r)
    p = np.arange(128)[:, None]
    c = np.arange(512)[None, :]
    masks = [(c >= p + 128 * r).astype(np.float32) for r in range(4)]
    mask = np.concatenate(masks, axis=1).astype(bf)  # [128, 2048]

    x = np.asarray(x)
    xts = [ktile(np.ascontiguousarray(x[b].T)) for b in range(B)]

    in_maps = []
    for core in range(N_CORES):
        b, g = core // 2, core % 2
        # wo row-slice for this head group, tiled by head: [128, 8, 2048]
        wo_rows = wo_s[g * NH * DH:(g + 1) * NH * DH, :]  # [1024, 2048]
        wo_t = np.ascontiguousarray(
            wo_rows.reshape(NH, 128, D).transpose(1, 0, 2)).astype(bf)
        in_maps.append({
            "xb": xts[b],
            "wq": ktile(np.asarray(Wq)[:, g * NH * DH:(g + 1) * NH * DH]),
            "wk": ktile(np.asarray(Wk)[:, g * NKV * DH:(g + 1) * NKV * DH]),
            "wv": ktile(np.asarray(Wv)[:, g * NKV * DH:(g + 1) * NKV * DH]),
            "wo": wo_t,
            "cost": cosT,
            "sint": sinT,
            "mask": mask,
        })
    return in_maps


def kernel(x, Wq, Wk, Wv, Wo, head_scale, _run_kwargs=None):
    nc = build_nc()
    in_maps = _host_prep(x, Wq, Wk, Wv, Wo, head_scale)
    res = run_bass_kernel_spmd(
        nc, in_maps, core_ids=list(range(N_CORES)), **(_run_kwargs or {})
    )
    # per-batch partial sums: core (b,0) + core (b,1)
    outs = []
    for b in range(B):
        p0 = res.results[2 * b]["out"].astype(np.float32)
        p1 = res.results[2 * b + 1]["out"].astype(np.float32)
        outs.append((p0 + p1).T)  # [1024, 2048]
    full = np.stack(outs, axis=0)  # [B, N, D]
    if _run_kwargs:
        kernel.last_results = res
    return full


# revision 4
# speedup vs baseline: 1.8049x; 1.0781x over previous
"""Trainium2 8-core kernel for causal GQA attention (nn_Attention_90967407329949).

Distribution: (batch x head-group) data parallel with host-side reduce — zero
device collectives. Core (b, g) handles batch b (1024 tokens) and q-heads
g*8..g*8+7 (plus their 2 kv heads): it computes Q/K/V projections for its
heads over the full batch, RoPE, full causal attention, and a PARTIAL output
projection (contraction over its 8 heads' dims of Wo). The host sums the two
partials per batch. Vs. tensor-parallel + AllGather this removes all
collectives (and their DMA-queue poisoning) and halves K/V projection work.

All matmuls run in bf16 (fp32 PSUM accumulation). head_scale is folded into Wo
rows on the host. Softmax skips the running-max (scores are O(1) here: |s|max
~ 4 after scaling, exp never overflows); denominators come from a ones-vector
matmul accumulated alongside the attention*V matmuls.

Schedule notes:
- K/V projections run kt-major with 7 concurrent PSUM accumulation groups so
  TensorE consumes x k-tiles no faster than the x DMA delivers them.
- RoPE reads the fp32 projection PSUM directly (PSUM+SBUF operand mix is
  exempt from the equal-base-partition rule, so rotate-half needs no DMA and
  Q/K need no PSUM->SBUF copy at all).
- V transposes + x + tables on the sync DMA queue; weights on the scalar
  queue.
- Attention for head h (ScalarE exp-heavy) interleaves with the Q projection
  of head h+1 (TensorE-bound) via generators to keep TensorE dense.

Layouts (feature dim on SBUF partitions):
  xb   [128, 16, 1024]  x^T k-tiles for this batch, bf16
  wq   [128, 16, 1024]  Wq k-tiles, this core's 8 heads
  wk/wv[128, 16, 256]   this core's 2 kv heads
  wo   [128, 8, 2048]   (head_scale-folded) Wo row-tiles for its 8 heads
  cost/sint [128, 1024] rotary tables transposed; sint sign-folded
  mask [128, 2048]      4 causal masks for the 4 diagonal offsets
  out  [2048, 1024]     partial (out @ Wo)^T for this batch, bf16
"""

import numpy as np
import ml_dtypes

import concourse.bacc as bacc
import concourse.mybir as mybir
import concourse.tile as tile
from concourse.bass_utils import run_bass_kernel_spmd

BF16 = mybir.dt.bfloat16
F32 = mybir.dt.float32

N_CORES = 8
B = 4
N = 1024           # sequence length per batch
D = 2048           # model dim
DH = 128           # head dim
KT = D // 128      # 16 contraction k-tiles
NH = 8             # q heads per core
NKV = 2            # kv heads per core
SCALE = 1.0 / np.sqrt(DH)

_NC_CACHE = {}


def build_nc():
    if "nc" in _NC_CACHE:
        return _NC_CACHE["nc"]
    nc = bacc.Bacc("TRN2", target_bir_lowering=False, debug=False, num_devices=N_CORES)

    xb = nc.dram_tensor("xb", [128, KT, N], BF16, kind="ExternalInput")
    wq = nc.dram_tensor("wq", [128, KT, NH * 128], BF16, kind="ExternalInput")
    wk = nc.dram_tensor("wk", [128, KT, NKV * 128], BF16, kind="ExternalInput")
    wv = nc.dram_tensor("wv", [128, KT, NKV * 128], BF16, kind="ExternalInput")
    wo = nc.dram_tensor("wo", [128, NH, D], BF16, kind="ExternalInput")
    cost = nc.dram_tensor("cost", [128, N], BF16, kind="ExternalInput")
    sint = nc.dram_tensor("sint", [128, N], BF16, kind="ExternalInput")
    mask = nc.dram_tensor("mask", [128, 2048], BF16, kind="ExternalInput")
    out = nc.dram_tensor("out", [D, N], BF16, kind="ExternalOutput")

    with tile.TileContext(nc) as tc:
        with (
            tc.tile_pool(name="const", bufs=1) as constp,
            tc.tile_pool(name="persist", bufs=1) as persist,
            tc.tile_pool(name="vraw", bufs=2) as vrawp,
            tc.tile_pool(name="rope", bufs=2) as ropep,
            tc.tile_pool(name="ep", bufs=4) as ep,
            tc.tile_pool(name="etmpp", bufs=2) as etmpp,
            tc.tile_pool(name="recipp", bufs=2) as recipp,
            tc.tile_pool(name="rbcp", bufs=2) as rbcp,
            tc.tile_pool(name="oobp", bufs=3) as oobp,
            tc.tile_pool(name="psacc", bufs=3, space="PSUM") as psacc,
            tc.tile_pool(name="pss", bufs=2, space="PSUM") as pss,
            tc.tile_pool(name="psu", bufs=2, space="PSUM") as psu,
            tc.tile_pool(name="pssum", bufs=1, space="PSUM") as pssum,
        ):
            # ---- constants ----
            xb_sb = constp.tile([128, KT, N], BF16)
            wq_sb = constp.tile([128, KT, NH * 128], BF16)
            wk_sb = constp.tile([128, KT, NKV * 128], BF16)
            wv_sb = constp.tile([128, KT, NKV * 128], BF16)
            wo_sb = constp.tile([128, NH, D], BF16)
            cos_sb = constp.tile([128, N], BF16)
            sin_sb = constp.tile([128, N], BF16)
            mask_sb = constp.tile([128, 2048], BF16)
            ones_sb = constp.tile([128, 1], BF16)
            # x on the sync queue, k-tile-granular (first projections start
            # as soon as k-tile 0 lands); tables right behind x (needed by
            # K RoPE ~15us / first attention ~25us)
            for kt in range(KT):
                nc.sync.dma_start(xb_sb[:, kt, :], xb[:, kt, :])
            nc.sync.dma_start(cos_sb[:], cost[:])
            nc.sync.dma_start(sin_sb[:], sint[:])
            nc.sync.dma_start(mask_sb[:], mask[:])
            # small weights first, then wq; wo trails (needed only ~170us in)
            nc.scalar.dma_start(wk_sb[:], wk[:])
            nc.scalar.dma_start(wv_sb[:], wv[:])
            for c in range(4):
                nc.scalar.dma_start(wq_sb[:, c * 4:(c + 1) * 4, :],
                                    wq[:, c * 4:(c + 1) * 4, :])
            nc.vector.memset(ones_sb[:], 1.0)

            def late_consts():
                for c in range(4):
                    nc.scalar.dma_start(wo_sb[:, c * 2:(c + 1) * 2, :],
                                        wo[:, c * 2:(c + 1) * 2, :])

            # ---- persistent per-core tensors ----
            k_sb = persist.tile([128, NKV * N], BF16)   # RoPE'd K, [d, kv*1024]
            v_sb = persist.tile([128, NKV * N], BF16)   # 8 [tok,128] tiles per kv
            q_sb = [persist.tile([128, N], BF16, name=f"q{h}_sb") for h in range(NH)]
            att_sb = [persist.tile([128, N], BF16, name=f"att{h}_sb")
                      for h in range(NH)]

            def rope_ps(ps, dst, c0):
                """RoPE a [128,512] fp32 PSUM projection tile into dst cols
                c0..c0+512. rotate_half reads PSUM at partition offset (the
                equal-base rule only applies when both inputs are SBUF);
                sin_sb is sign-folded (rows 0:64 hold -sin)."""
                sl = slice(c0, c0 + 512)
                t1 = ropep.tile([128, 512], BF16, tag="t1")
                t2 = ropep.tile([128, 512], BF16, tag="t2")
                nc.vector.tensor_mul(t1[:], ps[:], cos_sb[:, sl])
                nc.vector.tensor_mul(t2[0:64, :], ps[64:128, :],
                                     sin_sb[0:64, sl])
                nc.vector.tensor_mul(t2[64:128, :], ps[0:64, :],
                                     sin_sb[64:128, sl])
                nc.vector.tensor_add(dst[:, sl], t1[:], t2[:])

            def v_transpose(vraw, kv, c):
                for i in range(4):
                    tt = kv * 8 + c * 4 + i
                    nc.sync.dma_start_transpose(
                        v_sb[:, tt * 128:(tt + 1) * 128],
                        vraw[:, c * 512 + i * 128:c * 512 + (i + 1) * 128])

            def kv_gen():
                """K and V projections, kt-major with 7 concurrent PSUM
                groups (so TensorE never outruns the x DMA), + RoPE(K) from
                PSUM + transpose(V). Pool slots are ordered so fast-freed V
                tiles rotate to the next users first."""
                late_consts()
                vraw = [vrawp.tile([128, N], BF16, tag="vraw",
                                   name=f"vraw_{kv}") for kv in range(NKV)]
                specs = [  # (pool, tag, w, kv, c, kind)
                    (psacc, "psacc", wv_sb, 0, 0, "v"),
                    (psacc, "psacc", wv_sb, 0, 1, "v"),
                    (pss, "pss", wv_sb, 1, 0, "v"),
                    (psacc, "psacc", wk_sb, 0, 0, "k"),
                    (pss, "pss", wk_sb, 0, 1, "k"),
                    (psu, "psu", wk_sb, 1, 0, "k"),
                    (psu, "psu", wk_sb, 1, 1, "k"),
                ]
                tiles = [pool.tile([128, 512], F32, tag=tag,
                                   name=f"kv_ps_{kind}{kv}{c}")
                         for pool, tag, w, kv, c, kind in specs]
                for kt in range(KT):
                    for (pool, tag, w, kv, c, kind), ps in zip(specs, tiles):
                        nc.tensor.matmul(
                            ps[:], w[:, kt, kv * 128:(kv + 1) * 128],
                            xb_sb[:, kt, c * 512:(c + 1) * 512],
                            start=(kt == 0), stop=(kt == KT - 1))
                    yield
                # V evacuation first: scalar copies free PSUM fast, and the
                # transposes hit the sync queue early (x is done by now)
                for (pool, tag, w, kv, c, kind), ps in zip(specs, tiles):
                    if kind == "v":
                        nc.scalar.activation(vraw[kv][:, c * 512:(c + 1) * 512],
                                             ps[:],
                                             mybir.ActivationFunctionType.Copy)
                        v_transpose(vraw[kv], kv, c)
                yield
                # K RoPE straight out of PSUM (DVE)
                for (pool, tag, w, kv, c, kind), ps in zip(specs, tiles):
                    if kind == "k":
                        rope_ps(ps, k_sb[:, kv * N:(kv + 1) * N], c * 512)
                        yield
                # 8th group: V kv1 c1 (psacc slot freed by V00's copy)
                v8 = psacc.tile([128, 512], F32, tag="psacc", name="kv_ps_v11")
                for k0 in range(0, KT, 4):
                    for kt in range(k0, k0 + 4):
                        nc.tensor.matmul(
                            v8[:], wv_sb[:, kt, 128:256],
                            xb_sb[:, kt, 512:1024],
                            start=(kt == 0), stop=(kt == KT - 1))
                    yield
                nc.scalar.activation(vraw[1][:, 512:1024], v8[:],
                                     mybir.ActivationFunctionType.Copy)
                v_transpose(vraw[1], 1, 1)
                yield

            def qproj_gen(h):
                """Q projection + RoPE (from PSUM) for head h."""
                for c in range(2):
                    q_ps = psacc.tile([128, 512], F32, tag="psacc",
                                      name=f"q_ps_{h}_{c}")
                    for k0 in range(0, KT, 4):
                        for kt in range(k0, k0 + 4):
                            nc.tensor.matmul(
                                q_ps[:], wq_sb[:, kt, h * 128:(h + 1) * 128],
                                xb_sb[:, kt, c * 512:(c + 1) * 512],
                                start=(kt == 0), stop=(kt == KT - 1))
                        yield
                    rope_ps(q_ps, q_sb[h], c * 512)
                    yield

            def att_gen(h):
                """Causal attention for head h, yielding between j-tile units."""
                kv = h // 4
                qh = q_sb[h]
                att = att_sb[h]
                for ib in range(2):
                    icol = ib * 512
                    cnt = 4 * ib + 4
                    u_ps = psu.tile([128, 512], F32, tag="psu",
                                    name=f"u_ps_{h}_{ib}")
                    sum_ps = pssum.tile([1, 512], F32, tag="pssum",
                                        name=f"sum_ps_{h}_{ib}")

                    def c_lo(jt):
                        # diagonal tile at offset r: columns < 128*r are
                        # causally invalid for every row -- skip them in
                        # every consumer (exact: those (j,i) pairs are
                        # fully masked).
                        r = jt - 4 * ib
                        return 128 * r if r > 0 else 0

                    def s_mm(jt):
                        s_ps = pss.tile([128, 512], F32, tag="pss",
                                        name=f"s_ps_{h}_{ib}_{jt}")
                        jcol = kv * N + jt * 128
                        c0 = c_lo(jt)
                        nc.tensor.matmul(
                            s_ps[:, c0:512], k_sb[:, jcol:jcol + 128],
                            qh[:, icol + c0:icol + 512],
                            start=True, stop=True)
                        return s_ps

                    def e_of(jt, s_ps):
                        r = jt - 4 * ib
                        c0 = c_lo(jt)
                        e = ep.tile([128, 512], BF16, tag="e",
                                    name=f"e_{h}_{ib}_{jt}")
                        if r >= 0:  # diagonal tile: mask after exp
                            etmp = etmpp.tile([128, 512], BF16, tag="etmp")
                            nc.scalar.activation(
                                etmp[:, c0:512], s_ps[:, c0:512],
                                mybir.ActivationFunctionType.Exp, scale=SCALE)
                            nc.vector.tensor_mul(
                                e[:, c0:512], etmp[:, c0:512],
                                mask_sb[:, r * 512 + c0:(r + 1) * 512])
                        else:
                            nc.scalar.activation(
                                e[:], s_ps[:],
                                mybir.ActivationFunctionType.Exp, scale=SCALE)
                        return e

                    s_tiles = {0: s_mm(0), 1: s_mm(1)}
                    for jt in range(cnt):
                        e = e_of(jt, s_tiles.pop(jt))
                        if jt + 2 < cnt:
                            s_tiles[jt + 2] = s_mm(jt + 2)
                        tt = kv * 8 + jt
                        c0 = c_lo(jt)
                        nc.tensor.matmul(
                            u_ps[:, c0:512],
                            v_sb[:, tt * 128:(tt + 1) * 128], e[:, c0:512],
                            start=(jt == 0), stop=(jt == cnt - 1),
                            skip_group_check=True)
                        nc.tensor.matmul(
                            sum_ps[:, c0:512], ones_sb[:], e[:, c0:512],
                            start=(jt == 0), stop=(jt == cnt - 1),
                            skip_group_check=True)
                        yield
                    recip = recipp.tile([1, 512], F32, tag="recip")
                    nc.vector.reciprocal_approx_fast(out=recip[:], in_=sum_ps[:])
                    rbc = rbcp.tile([128, 512], F32, tag="rbc")
                    nc.gpsimd.partition_broadcast(rbc[:], recip[:])
                    nc.vector.tensor_mul(
                        att[:, ib * 512:(ib + 1) * 512], u_ps[:], rbc[:])
                    yield

            def oproj_gen():
                """Partial output projection: contraction over this core's
                8 heads; DMA out each [128,512] half as it is ready."""
                for m in range(16):
                    osb = oobp.tile([128, N], BF16, tag="osb", name=f"osb_{m}")
                    for c in range(2):
                        o_ps = psacc.tile([128, 512], F32, tag="psacc",
                                          name=f"o_ps_{m}_{c}")
                        for hh in range(NH):
                            nc.tensor.matmul(
                                o_ps[:], wo_sb[:, hh, m * 128:(m + 1) * 128],
                                att_sb[hh][:, c * 512:(c + 1) * 512],
                                start=(hh == 0), stop=(hh == NH - 1))
                            if hh % 4 == 3:
                                yield
                        nc.scalar.activation(osb[:, c * 512:(c + 1) * 512],
                                             o_ps[:],
                                             mybir.ActivationFunctionType.Copy)
                        nc.sync.dma_start(
                            out[m * 128:(m + 1) * 128, c * 512:(c + 1) * 512],
                            osb[:, c * 512:(c + 1) * 512])
                        yield

            def drain(gen):
                for _ in gen:
                    pass

            def interleave(gen_a, gen_b, ratio_a=1):
                alive = [gen_a, gen_b]
                while alive:
                    for g in list(alive):
                        steps = ratio_a if g is gen_a else 1
                        for _ in range(steps):
                            try:
                                next(g)
                            except StopIteration:
                                if g in alive:
                                    alive.remove(g)
                                break

            # Pipeline: K/V projections kt-major; attention(h) (ScalarE-heavy)
            # interleaves with Q projection of head h+1 (TensorE-bound);
            # output projection is one dense matmul run at the end.
            drain(kv_gen())
            drain(qproj_gen(0))
            for h in range(NH):
                if h + 1 < NH:
                    interleave(att_gen(h), qproj_gen(h + 1))
                else:
                    drain(att_gen(h))
            drain(oproj_gen())

    nc.compile()
    _NC_CACHE["nc"] = nc
    return nc


def _host_prep(x, Wq, Wk, Wv, Wo, head_scale):
    bf = ml_dtypes.bfloat16

    hs = np.asarray(head_scale).reshape(16)
    wo_s = (np.asarray(Wo) * np.repeat(hs, DH)[:, None]).astype(np.float32)

    def ktile(w):  # [2048, M] -> [128, 16, M]
        m = w.shape[1]
        return np.ascontiguousarray(
            w.reshape(KT, 128, m).transpose(1, 0, 2)).astype(bf)

    inv_freq = (1.0 / (10000.0 ** (np.arange(0, DH, 2, dtype=np.float64) / DH)))
    freqs = np.arange(N, dtype=np.float64)[:, None] * inv_freq[None, :]  # [N, 64]
    emb = np.concatenate([freqs, freqs], axis=-1)  # [N, 128]
    cosT = np.ascontiguousarray(np.cos(emb).T).astype(bf)  # [128, N]
    sinT = np.sin(emb).T  # [128, N]
    sign = np.where(np.arange(DH) < 64, -1.0, 1.0)[:, None]
    sinT = np.ascontiguousarray(sinT * sign).astype(bf)

    # 4 diagonal masks r=0..3: valid (c >= p + 128*r)
    p = np.arange(128)[:, None]
    c = np.arange(512)[None, :]
    masks = [(c >= p + 128 * r).astype(np.float32) for r in range(4)]
    mask = np.concatenate(masks, axis=1).astype(bf)  # [128, 2048]

    x = np.asarray(x)
    xts = [ktile(np.ascontiguousarray(x[b].T)) for b in range(B)]

    in_maps = []
    for core in range(N_CORES):
        b, g = core // 2, core % 2
        # wo row-slice for this head group, tiled by head: [128, 8, 2048]
        wo_rows = wo_s[g * NH * DH:(g + 1) * NH * DH, :]  # [1024, 2048]
        wo_t = np.ascontiguousarray(
            wo_rows.reshape(NH, 128, D).transpose(1, 0, 2)).astype(bf)
        in_maps.append({
            "xb": xts[b],
            "wq": ktile(np.asarray(Wq)[:, g * NH * DH:(g + 1) * NH * DH]),
            "wk": ktile(np.asarray(Wk)[:, g * NKV * DH:(g + 1) * NKV * DH]),
            "wv": ktile(np.asarray(Wv)[:, g * NKV * DH:(g + 1) * NKV * DH]),
            "wo": wo_t,
            "cost": cosT,
            "sint": sinT,
            "mask": mask,
        })
    return in_maps


def kernel(x, Wq, Wk, Wv, Wo, head_scale, _run_kwargs=None):
    nc = build_nc()
    in_maps = _host_prep(x, Wq, Wk, Wv, Wo, head_scale)
    res = run_bass_kernel_spmd(
        nc, in_maps, core_ids=list(range(N_CORES)), **(_run_kwargs or {})
    )
    # per-batch partial sums: core (b,0) + core (b,1)
    outs = []
    for b in range(B):
        p0 = res.results[2 * b]["out"].astype(np.float32)
        p1 = res.results[2 * b + 1]["out"].astype(np.float32)
        outs.append((p0 + p1).T)  # [1024, 2048]
    full = np.stack(outs, axis=0)  # [B, N, D]
    if _run_kwargs:
        kernel.last_results = res
    return full
